# revision 30
# baseline (speedup 1.0000x reference)
"""MultiHeadCrossAttention on 8 TRN2 NeuronCores.

Sharding: tensor-parallel over heads (16 heads -> 2 per core).
All activations live transposed ([features, tokens]) on device so every
matmul contracts over the partition dim with zero on-device transposes of
the big activations (V is PE-transposed per 128-col block, which is cheap).
Per core:
  Q.T = (Wq.T slice).T @ x1.T   [128, 4096]
  K.T, V.T from x2.T            [128, 8192]
  per (batch, qcol-chunk, head): S.T = K @ Q.T ; P.T = exp(S.T/8) ;
    outT[d|den] = [V|1]-chunks.T @ P.T  (ones column gives the softmax
    denominator for free) ; attnT = outT[0:64] * recip(outT[64])
  Y.T partial = (Wo.T row-slice).T @ attnT  [1024, 4096]
Host: pre-tiles inputs for contiguous DMA, sums the 8 partials, adds bo,
transposes back. Emission is software-pipelined: KV-projection of batch
b+1 is emitted before attention of batch b; out-projection is fused per
q-column chunk right after its normalize.
"""
import numpy as np
from contextlib import ExitStack

import concourse.bass as bass
import concourse.mybir as mybir
import concourse.tile as tile
from concourse import bacc
from concourse.bass_utils import run_bass_kernel_spmd

N_CORES = 8
B, SQ, SKV, E, DH = 4, 1024, 2048, 1024, 64
Q_ROWS = B * SQ      # 4096
KV_ROWS = B * SKV    # 8192
EC = E // 128        # 8 contraction chunks
QC = Q_ROWS // 512   # 8 q column chunks
KVC_B = SKV // 128   # 16 kv chunks per batch
GB = SQ // 512       # 2 q chunks per batch
F32R = mybir.dt.float32r
F32 = mybir.dt.float32
Exp = mybir.ActivationFunctionType.Exp

_CACHE = {}


def _build(phases=("proj", "attn", "oproj"), n_reps=1):
    nc = bacc.Bacc("TRN2", target_bir_lowering=False, debug=False,
                   num_devices=N_CORES)
    # host-pretiled inputs: each [.., 128, EC, 512] slab is one contiguous DMA
    x1t = nc.dram_tensor("x1t", [QC, 128, EC, 512], F32R,
                         kind="ExternalInput").ap()
    x2t = nc.dram_tensor("x2t", [KV_ROWS // 512, 128, EC, 512], F32R,
                         kind="ExternalInput").ap()
    wqt = nc.dram_tensor("wqt", [128, EC, 128], F32R, kind="ExternalInput").ap()
    wkt = nc.dram_tensor("wkt", [128, EC, 128], F32R, kind="ExternalInput").ap()
    wvt = nc.dram_tensor("wvt", [128, EC, 128], F32R, kind="ExternalInput").ap()
    wot = nc.dram_tensor("wot", [128, E], F32R, kind="ExternalInput").ap()
    bqv = nc.dram_tensor("bq", [128, 1], F32, kind="ExternalInput").ap()
    bkv = nc.dram_tensor("bk", [128, 1], F32, kind="ExternalInput").ap()
    bvv = nc.dram_tensor("bv", [128, 1], F32, kind="ExternalInput").ap()
    idv = nc.dram_tensor("ident", [128, 128], F32R, kind="ExternalInput").ap()
    onv = nc.dram_tensor("ones", [128, 1], F32R, kind="ExternalInput").ap()
    yt = nc.dram_tensor("yt", [E, Q_ROWS], F32, kind="ExternalOutput").ap()
    yt_r = yt.rearrange("(oc p) q -> p oc q", p=128)

    do_proj = "proj" in phases
    do_attn = "attn" in phases and do_proj
    do_oproj = "oproj" in phases and do_attn

    with tile.TileContext(nc) as tc, ExitStack() as ctx:
        const = ctx.enter_context(tc.tile_pool(name="const", bufs=1))
        persist = ctx.enter_context(tc.tile_pool(name="persist", bufs=1))
        xload = ctx.enter_context(tc.tile_pool(name="xload", bufs=8))
        work = ctx.enter_context(tc.tile_pool(name="work", bufs=3))
        ps_pj = ctx.enter_context(tc.tile_pool(name="ps_pj", bufs=2, space="PSUM"))
        ps_s = ctx.enter_context(tc.tile_pool(name="ps_s", bufs=2, space="PSUM"))
        ps_o = ctx.enter_context(tc.tile_pool(name="ps_o", bufs=2, space="PSUM"))

        wq_sb = const.tile([128, EC, 128], F32R, tag="wq")
        wk_sb = const.tile([128, EC, 128], F32R, tag="wk")
        wv_sb = const.tile([128, EC, 128], F32R, tag="wv")
        wo_sb = const.tile([128, E], F32R, tag="wo")
        bq_sb = const.tile([128, 1], F32, tag="bq")
        bk_sb = const.tile([128, 1], F32, tag="bk")
        bv_sb = const.tile([128, 1], F32, tag="bv")
        id_sb = const.tile([128, 128], F32R, tag="id")
        ones_sb = const.tile([128, 1], F32R, tag="ones1")
        nc.sync.dma_start(wq_sb[:], wqt[:])
        nc.sync.dma_start(wk_sb[:], wkt[:])
        nc.sync.dma_start(wv_sb[:], wvt[:])
        nc.sync.dma_start(wo_sb[:], wot[:])
        nc.sync.dma_start(bq_sb[:], bqv[:])
        nc.sync.dma_start(bk_sb[:], bkv[:])
        nc.sync.dma_start(bv_sb[:], bvv[:])
        nc.sync.dma_start(id_sb[:], idv[:])
        nc.sync.dma_start(ones_sb[:], onv[:])

        for rep in range(n_reps):
            qt_sb = persist.tile([128, Q_ROWS], F32R, tag="qt", name=f"qt_{rep}")
            kt_sb = [persist.tile([128, SKV], F32R, tag=f"kt{b}",
                                  name=f"kt{b}_{rep}") for b in range(B)]
            v_sb = [persist.tile([128, KVC_B, 130], F32R, tag=f"v{b}",
                                 name=f"v{b}_{rep}") for b in range(B)]
            at_sb = [persist.tile([128, SQ], F32R, tag=f"at{b}",
                                  name=f"atz{b}_{rep}") for b in range(B)]

            def proj_q(j):
                for u in range(2):
                    xt = xload.tile([128, EC, 256], F32R, tag="x",
                                    name=f"xq{j}_{u}_{rep}")
                    nc.sync.dma_start(xt[:], x1t[j][:, :, u * 256:(u + 1) * 256])
                    if not do_proj:
                        continue
                    q_ps = ps_pj.tile([128, 256], F32, tag="pj",
                                      name=f"qps{j}_{u}_{rep}")
                    for ec in range(EC):
                        nc.tensor.matmul(q_ps[:], wq_sb[:, ec], xt[:, ec],
                                         start=(ec == 0), stop=(ec == EC - 1))
                    c0 = j * 512 + u * 256
                    nc.vector.tensor_scalar_add(qt_sb[:, c0:c0 + 256],
                                                q_ps[:], bq_sb[:])

            def proj_kv(b, half=None):
                rng = range(SKV // 512) if half is None else \
                    range(half * (SKV // 1024), (half + 1) * (SKV // 1024))
                for jj in rng:
                    j = b * (SKV // 512) + jj
                    for u in range(2):
                        xt = xload.tile([128, EC, 256], F32R, tag="x",
                                        name=f"xt{b}_{jj}_{u}_{rep}")
                        nc.sync.dma_start(xt[:],
                                          x2t[j][:, :, u * 256:(u + 1) * 256])
                        if not do_proj:
                            continue
                        k_ps = ps_pj.tile([128, 256], F32, tag="pj",
                                          name=f"kps{b}_{jj}_{u}_{rep}")
                        for ec in range(EC):
                            nc.tensor.matmul(k_ps[:], wk_sb[:, ec], xt[:, ec],
                                             start=(ec == 0), stop=(ec == EC - 1))
                        c0 = jj * 512 + u * 256
                        nc.vector.tensor_scalar_add(
                            kt_sb[b][:, c0:c0 + 256], k_ps[:], bk_sb[:])
                        v_ps = ps_pj.tile([128, 256], F32, tag="pj",
                                          name=f"vps{b}_{jj}_{u}_{rep}")
                        for ec in range(EC):
                            nc.tensor.matmul(v_ps[:], wv_sb[:, ec], xt[:, ec],
                                             start=(ec == 0), stop=(ec == EC - 1))
                        vt_tmp = work.tile([128, 256], F32R, tag="vt", bufs=2,
                                           name=f"vtt{b}_{jj}_{u}_{rep}")
                        nc.vector.tensor_scalar_add(vt_tmp[:], v_ps[:], bv_sb[:])
                        for t in range(2):
                            kc = jj * 4 + u * 2 + t
                            vtp = ps_pj.tile([128, 128], F32R, tag="pj",
                                             name=f"vtp{b}_{kc}_{rep}")
                            nc.tensor.transpose(vtp[:],
                                                vt_tmp[:, t * 128:(t + 1) * 128],
                                                id_sb[:])
                            dst = v_sb[b][:, kc].rearrange("p (h x) -> p h x",
                                                           h=2)
                            nc.vector.tensor_copy(
                                dst[:, :, 0:64],
                                vtp[:].rearrange("p (h x) -> p h x", h=2))

            def oproj_g(b, g):
                if not do_oproj:
                    return
                for o in range(EC):
                    y_ps = ps_pj.tile([128, 512], F32, tag="pj",
                                      name=f"yps{b}_{g}_{o}_{rep}")
                    nc.tensor.matmul(y_ps[:], wo_sb[:, o * 128:(o + 1) * 128],
                                     at_sb[b][:, g * 512:(g + 1) * 512],
                                     start=True, stop=True)
                    y_sb = work.tile([128, 512], F32, tag="y", bufs=2,
                                     name=f"ysb{b}_{g}_{o}_{rep}")
                    nc.vector.tensor_copy(y_sb[:], y_ps[:])
                    nc.sync.dma_start(
                        yt_r[:, o, b * SQ + g * 512: b * SQ + (g + 1) * 512],
                        y_sb[:])

            def attn(b, gsel=None):
                if not do_attn:
                    return
                if gsel in (None, 0):
                    vv = v_sb[b][:].rearrange("p kc (h x) -> p (kc h) x", x=65)
                    nc.vector.tensor_copy(vv[:, :, 64:65],
                                          ones_sb[:].unsqueeze(-1)
                                          .to_broadcast((128, 2 * KVC_B, 1)))
                for g in range(GB) if gsel is None else [gsel]:
                    gs = slice(g * 512, (g + 1) * 512)
                    o_ps = [ps_o.tile([65, 512], F32, tag="o",
                                      name=f"o{b}_{g}_{h}_{rep}")
                            for h in range(2)]
                    for kc in range(0, KVC_B, 2):
                        for h in range(2):
                            hp = h * 64
                            s_ps = ps_s.tile([128, 1024], F32, tag="s",
                                             name=f"sps{b}_{g}_{kc}_{h}_{rep}")
                            pt = work.tile([128, 1024], F32R, tag="pt", bufs=4,
                                           name=f"pt{b}_{g}_{kc}_{h}_{rep}")
                            for u in range(2):
                                nc.tensor.matmul(
                                    s_ps[:, u * 512:(u + 1) * 512],
                                    kt_sb[b][hp:hp + 64,
                                             (kc + u) * 128:(kc + u + 1) * 128],
                                    qt_sb[hp:hp + 64, b * SQ + g * 512:
                                          b * SQ + (g + 1) * 512],
                                    start=True, stop=True)
                            nc.scalar.activation(pt[:], s_ps[:], Exp,
                                                 scale=0.125)
                            for u in range(2):
                                nc.tensor.matmul(
                                    o_ps[h][:],
                                    v_sb[b][:, kc + u, h * 65:h * 65 + 65],
                                    pt[:, u * 512:(u + 1) * 512],
                                    start=(kc == 0 and u == 0),
                                    stop=(kc == KVC_B - 2 and u == 1))
                    for h in range(2):
                        hp = h * 64
                        recip = work.tile([1, 512], F32, tag="recip", bufs=2,
                                          name=f"rc{b}_{g}_{h}_{rep}")
                        nc.vector.reciprocal(recip[:], o_ps[h][64:65, :])
                        rbc = work.tile([64, 512], F32, tag="rbc", bufs=2,
                                        name=f"rbc{b}_{g}_{h}_{rep}")
                        nc.gpsimd.partition_broadcast(rbc[:], recip[:])
                        nc.vector.tensor_mul(at_sb[b][hp:hp + 64, gs],
                                             o_ps[h][0:64, :], rbc[:])
                    oproj_g(b, g)

            # software-pipelined emission: proj(b+1) ahead of attn(b),
            # Q chunks just-in-time (attn(b) needs chunks 2b, 2b+1)
            proj_q(0)
            proj_q(1)
            proj_kv(0)
            for b in range(B):
                if b + 1 < B:
                    proj_q(2 * b + 2)
                    proj_kv(b + 1, half=0)
                    attn(b, gsel=0)
                    proj_q(2 * b + 3)
                    proj_kv(b + 1, half=1)
                    attn(b, gsel=1)
                else:
                    attn(b)

    nc.compile()
    return nc


def _get_nc(phases=("proj", "attn", "oproj"), n_reps=1):
    key = (tuple(phases), n_reps)
    if key not in _CACHE:
        _CACHE[key] = _build(phases, n_reps)
    return _CACHE[key]


def _tile_x(xt2d, nchunks):
    # [E, R] -> [R/512, 128, EC, 512]: x[j, p, ec, q] = xt2d[ec*128+p, j*512+q]
    return np.ascontiguousarray(
        xt2d.reshape(EC, 128, nchunks, 512).transpose(2, 1, 0, 3))


def _tile_w(wt_slice):
    # [E, 128] -> [128, EC, 128]
    return np.ascontiguousarray(
        wt_slice.reshape(EC, 128, 128).transpose(1, 0, 2))


def make_in_maps(x1, x2, Wq, bq, Wk, bk, Wv, bv, Wo, bo=None):
    x1 = np.asarray(x1, dtype=np.float32)
    x2 = np.asarray(x2, dtype=np.float32)
    x1t = _tile_x(np.ascontiguousarray(x1.reshape(Q_ROWS, E).T), QC)
    x2t = _tile_x(np.ascontiguousarray(x2.reshape(KV_ROWS, E).T),
                  KV_ROWS // 512)
    WqT = np.asarray(Wq, dtype=np.float32).T
    WkT = np.asarray(Wk, dtype=np.float32).T
    WvT = np.asarray(Wv, dtype=np.float32).T
    WoT = np.ascontiguousarray(np.asarray(Wo, dtype=np.float32).T)
    ident = np.eye(128, dtype=np.float32)
    ones = np.ones((128, 1), dtype=np.float32)
    in_maps = []
    for c in range(N_CORES):
        s = slice(128 * c, 128 * (c + 1))
        in_maps.append({
            "x1t": x1t, "x2t": x2t,
            "wqt": _tile_w(WqT[:, s]),
            "wkt": _tile_w(WkT[:, s]),
            "wvt": _tile_w(WvT[:, s]),
            "wot": np.ascontiguousarray(WoT[s, :]),
            "bq": np.ascontiguousarray(
                np.asarray(bq, np.float32)[s]).reshape(128, 1),
            "bk": np.ascontiguousarray(
                np.asarray(bk, np.float32)[s]).reshape(128, 1),
            "bv": np.ascontiguousarray(
                np.asarray(bv, np.float32)[s]).reshape(128, 1),
            "ident": ident, "ones": ones,
        })
    return in_maps


def kernel(x1, x2, Wq, bq, Wk, bk, Wv, bv, Wo, bo):
    nc = _get_nc()
    in_maps = make_in_maps(x1, x2, Wq, bq, Wk, bk, Wv, bv, Wo)
    res = run_bass_kernel_spmd(nc, in_maps, list(range(N_CORES)))
    ytf = res.results[0]["yt"].astype(np.float64)
    for c in range(1, N_CORES):
        ytf += res.results[c]["yt"]
    y = ytf.T.astype(np.float32) + np.asarray(bo, np.float32)[None, :]
    return y.reshape(B, SQ, E)


# revision 34
# speedup vs baseline: 1.0503x; 1.0503x over previous
"""MultiHeadCrossAttention on 8 TRN2 NeuronCores.

Sharding: tensor-parallel over heads (16 heads -> 2 per core).
All activations live transposed ([features, tokens]) on device so every
matmul contracts over the partition dim with zero on-device transposes of
the big activations (V is PE-transposed per 128-col block, which is cheap).
Per core:
  Q.T = (Wq.T slice).T @ x1.T   [128, 4096]
  K.T, V.T from x2.T            [128, 8192]
  per (batch, qcol-chunk, head): S.T = K @ Q.T ; P.T = exp(S.T/8) ;
    outT[d|den] = [V|1]-chunks.T @ P.T  (ones column gives the softmax
    denominator for free) ; attnT = outT[0:64] * recip(outT[64])
  Y.T partial = (Wo.T row-slice).T @ attnT  [1024, 4096]
Host: pre-tiles inputs for contiguous DMA, sums the 8 partials, adds bo,
transposes back. Emission is software-pipelined: KV-projection of batch
b+1 is emitted before attention of batch b; out-projection is fused per
q-column chunk right after its normalize.
"""
import numpy as np
from contextlib import ExitStack

import concourse.bass as bass
import concourse.mybir as mybir
import concourse.tile as tile
from concourse import bacc
from concourse.bass_utils import run_bass_kernel_spmd

N_CORES = 8
B, SQ, SKV, E, DH = 4, 1024, 2048, 1024, 64
Q_ROWS = B * SQ      # 4096
KV_ROWS = B * SKV    # 8192
EC = E // 128        # 8 contraction chunks
QC = Q_ROWS // 512   # 8 q column chunks
KVC_B = SKV // 128   # 16 kv chunks per batch
GB = SQ // 512       # 2 q chunks per batch
F32R = mybir.dt.float32r
F32 = mybir.dt.float32
Exp = mybir.ActivationFunctionType.Exp

_CACHE = {}


def _build(phases=("proj", "attn", "oproj"), n_reps=1):
    nc = bacc.Bacc("TRN2", target_bir_lowering=False, debug=False,
                   num_devices=N_CORES)
    # host-pretiled inputs: each [.., 128, EC, 512] slab is one contiguous DMA
    x1t = nc.dram_tensor("x1t", [QC, 128, EC, 512], F32R,
                         kind="ExternalInput").ap()
    x2t = nc.dram_tensor("x2t", [KV_ROWS // 512, 128, EC, 512], F32R,
                         kind="ExternalInput").ap()
    wqt = nc.dram_tensor("wqt", [128, EC, 128], F32R, kind="ExternalInput").ap()
    wkt = nc.dram_tensor("wkt", [128, EC, 128], F32R, kind="ExternalInput").ap()
    wvt = nc.dram_tensor("wvt", [128, EC, 128], F32R, kind="ExternalInput").ap()
    wot = nc.dram_tensor("wot", [128, E], F32R, kind="ExternalInput").ap()
    bqv = nc.dram_tensor("bq", [128, 1], F32, kind="ExternalInput").ap()
    bkv = nc.dram_tensor("bk", [128, 1], F32, kind="ExternalInput").ap()
    bvv = nc.dram_tensor("bv", [128, 1], F32, kind="ExternalInput").ap()
    idv = nc.dram_tensor("ident", [128, 128], F32R, kind="ExternalInput").ap()
    onv = nc.dram_tensor("ones", [128, 1], F32R, kind="ExternalInput").ap()
    yt = nc.dram_tensor("yt", [E, Q_ROWS], F32, kind="ExternalOutput").ap()
    yt_r = yt.rearrange("(oc p) q -> p oc q", p=128)

    do_proj = "proj" in phases
    do_attn = "attn" in phases and do_proj
    do_oproj = "oproj" in phases and do_attn

    with tile.TileContext(nc) as tc, ExitStack() as ctx:
        const = ctx.enter_context(tc.tile_pool(name="const", bufs=1))
        persist = ctx.enter_context(tc.tile_pool(name="persist", bufs=1))
        xload = ctx.enter_context(tc.tile_pool(name="xload", bufs=7))
        work = ctx.enter_context(tc.tile_pool(name="work", bufs=3))
        ps_pj = ctx.enter_context(tc.tile_pool(name="ps_pj", bufs=2, space="PSUM"))
        ps_s = ctx.enter_context(tc.tile_pool(name="ps_s", bufs=2, space="PSUM"))
        ps_o = ctx.enter_context(tc.tile_pool(name="ps_o", bufs=2, space="PSUM"))

        wq_sb = const.tile([128, EC, 128], F32R, tag="wq")
        wk_sb = const.tile([128, EC, 128], F32R, tag="wk")
        wv_sb = const.tile([128, EC, 128], F32R, tag="wv")
        wo_sb = const.tile([128, E], F32R, tag="wo")
        bq_sb = const.tile([128, 1], F32, tag="bq")
        bk_sb = const.tile([128, 1], F32, tag="bk")
        bv_sb = const.tile([128, 1], F32, tag="bv")
        id_sb = const.tile([128, 128], F32R, tag="id")
        ones_sb = const.tile([128, 1], F32R, tag="ones1")
        nc.sync.dma_start(wq_sb[:], wqt[:])
        nc.sync.dma_start(wk_sb[:], wkt[:])
        nc.sync.dma_start(wv_sb[:], wvt[:])
        nc.sync.dma_start(wo_sb[:], wot[:])
        nc.sync.dma_start(bq_sb[:], bqv[:])
        nc.sync.dma_start(bk_sb[:], bkv[:])
        nc.sync.dma_start(bv_sb[:], bvv[:])
        nc.sync.dma_start(id_sb[:], idv[:])
        nc.sync.dma_start(ones_sb[:], onv[:])

        for rep in range(n_reps):
            qt_sb = persist.tile([128, Q_ROWS], F32R, tag="qt", name=f"qt_{rep}")
            kt_sb = [persist.tile([128, SKV], F32R, tag=f"kt{b}",
                                  name=f"kt{b}_{rep}") for b in range(B)]
            v_sb = [persist.tile([128, KVC_B, 130], F32R, tag=f"v{b}",
                                 name=f"v{b}_{rep}") for b in range(B)]
            at_sb = [persist.tile([128, SQ], F32R, tag=f"at{b}",
                                  name=f"atz{b}_{rep}") for b in range(B)]

            def proj_q(j):
                for u in range(2):
                    xt = xload.tile([128, EC, 256], F32R, tag="x",
                                    name=f"xq{j}_{u}_{rep}")
                    nc.sync.dma_start(xt[:], x1t[j][:, :, u * 256:(u + 1) * 256])
                    if not do_proj:
                        continue
                    q_ps = ps_pj.tile([128, 256], F32, tag="pj",
                                      name=f"qps{j}_{u}_{rep}")
                    for ec in range(EC):
                        nc.tensor.matmul(q_ps[:], wq_sb[:, ec], xt[:, ec],
                                         start=(ec == 0), stop=(ec == EC - 1))
                    c0 = j * 512 + u * 256
                    nc.vector.tensor_scalar_add(qt_sb[:, c0:c0 + 256],
                                                q_ps[:], bq_sb[:])

            def proj_kv(b, half=None):
                rng = range(SKV // 512) if half is None else \
                    range(half * (SKV // 1024), (half + 1) * (SKV // 1024))
                for jj in rng:
                    j = b * (SKV // 512) + jj
                    for u in range(2):
                        xt = xload.tile([128, EC, 256], F32R, tag="x",
                                        name=f"xt{b}_{jj}_{u}_{rep}")
                        nc.sync.dma_start(xt[:],
                                          x2t[j][:, :, u * 256:(u + 1) * 256])
                        if not do_proj:
                            continue
                        k_ps = ps_pj.tile([128, 256], F32, tag="pj",
                                          name=f"kps{b}_{jj}_{u}_{rep}")
                        for ec in range(EC):
                            nc.tensor.matmul(k_ps[:], wk_sb[:, ec], xt[:, ec],
                                             start=(ec == 0), stop=(ec == EC - 1))
                        c0 = jj * 512 + u * 256
                        nc.vector.tensor_scalar_add(
                            kt_sb[b][:, c0:c0 + 256], k_ps[:], bk_sb[:])
                        v_ps = ps_pj.tile([128, 256], F32, tag="pj",
                                          name=f"vps{b}_{jj}_{u}_{rep}")
                        for ec in range(EC):
                            nc.tensor.matmul(v_ps[:], wv_sb[:, ec], xt[:, ec],
                                             start=(ec == 0), stop=(ec == EC - 1))
                        vt_tmp = work.tile([128, 256], F32R, tag="vt", bufs=3,
                                           name=f"vtt{b}_{jj}_{u}_{rep}")
                        nc.vector.tensor_scalar_add(vt_tmp[:], v_ps[:], bv_sb[:])
                        for t in range(2):
                            kc = jj * 4 + u * 2 + t
                            vtp = ps_pj.tile([128, 128], F32R, tag="pj",
                                             name=f"vtp{b}_{kc}_{rep}")
                            nc.tensor.transpose(vtp[:],
                                                vt_tmp[:, t * 128:(t + 1) * 128],
                                                id_sb[:])
                            dst = v_sb[b][:, kc].rearrange("p (h x) -> p h x",
                                                           h=2)
                            nc.vector.tensor_copy(
                                dst[:, :, 0:64],
                                vtp[:].rearrange("p (h x) -> p h x", h=2))

            def oproj_g(b, g):
                if not do_oproj:
                    return
                for o in range(EC):
                    y_ps = ps_pj.tile([128, 512], F32, tag="pj",
                                      name=f"yps{b}_{g}_{o}_{rep}")
                    nc.tensor.matmul(y_ps[:], wo_sb[:, o * 128:(o + 1) * 128],
                                     at_sb[b][:, g * 512:(g + 1) * 512],
                                     start=True, stop=True)
                    y_sb = work.tile([128, 512], F32, tag="y", bufs=3,
                                     name=f"ysb{b}_{g}_{o}_{rep}")
                    nc.vector.tensor_copy(y_sb[:], y_ps[:])
                    nc.sync.dma_start(
                        yt_r[:, o, b * SQ + g * 512: b * SQ + (g + 1) * 512],
                        y_sb[:])

            def attn(b, gsel=None):
                if not do_attn:
                    return
                if gsel in (None, 0):
                    vv = v_sb[b][:].rearrange("p kc (h x) -> p (kc h) x", x=65)
                    nc.vector.tensor_copy(vv[:, :, 64:65],
                                          ones_sb[:].unsqueeze(-1)
                                          .to_broadcast((128, 2 * KVC_B, 1)))
                for g in range(GB) if gsel is None else [gsel]:
                    gs = slice(g * 512, (g + 1) * 512)
                    o_ps = [ps_o.tile([65, 512], F32, tag="o",
                                      name=f"o{b}_{g}_{h}_{rep}")
                            for h in range(2)]
                    for kc in range(0, KVC_B, 2):
                        for h in range(2):
                            hp = h * 64
                            s_ps = ps_s.tile([128, 1024], F32, tag="s",
                                             name=f"sps{b}_{g}_{kc}_{h}_{rep}")
                            pt = work.tile([128, 1024], F32R, tag="pt", bufs=4,
                                           name=f"pt{b}_{g}_{kc}_{h}_{rep}")
                            for u in range(2):
                                nc.tensor.matmul(
                                    s_ps[:, u * 512:(u + 1) * 512],
                                    kt_sb[b][hp:hp + 64,
                                             (kc + u) * 128:(kc + u + 1) * 128],
                                    qt_sb[hp:hp + 64, b * SQ + g * 512:
                                          b * SQ + (g + 1) * 512],
                                    start=True, stop=True)
                            nc.scalar.activation(pt[:], s_ps[:], Exp,
                                                 scale=0.125)
                            for u in range(2):
                                nc.tensor.matmul(
                                    o_ps[h][:],
                                    v_sb[b][:, kc + u, h * 65:h * 65 + 65],
                                    pt[:, u * 512:(u + 1) * 512],
                                    start=(kc == 0 and u == 0),
                                    stop=(kc == KVC_B - 2 and u == 1))
                    for h in range(2):
                        hp = h * 64
                        recip = work.tile([1, 512], F32, tag="recip", bufs=2,
                                          name=f"rc{b}_{g}_{h}_{rep}")
                        nc.vector.reciprocal(recip[:], o_ps[h][64:65, :])
                        rbc = work.tile([64, 512], F32, tag="rbc", bufs=2,
                                        name=f"rbc{b}_{g}_{h}_{rep}")
                        nc.gpsimd.partition_broadcast(rbc[:], recip[:])
                        nc.vector.tensor_mul(at_sb[b][hp:hp + 64, gs],
                                             o_ps[h][0:64, :], rbc[:])
                    oproj_g(b, g)

            # software-pipelined emission: proj(b+1) ahead of attn(b),
            # Q chunks just-in-time (attn(b) needs chunks 2b, 2b+1)
            proj_q(0)
            proj_q(1)
            proj_kv(0)
            for b in range(B):
                if b + 1 < B:
                    proj_q(2 * b + 2)
                    proj_kv(b + 1, half=0)
                    attn(b, gsel=0)
                    proj_q(2 * b + 3)
                    proj_kv(b + 1, half=1)
                    attn(b, gsel=1)
                else:
                    attn(b)

    nc.compile()
    return nc


def _get_nc(phases=("proj", "attn", "oproj"), n_reps=1):
    key = (tuple(phases), n_reps)
    if key not in _CACHE:
        _CACHE[key] = _build(phases, n_reps)
    return _CACHE[key]


def _tile_x(xt2d, nchunks):
    # [E, R] -> [R/512, 128, EC, 512]: x[j, p, ec, q] = xt2d[ec*128+p, j*512+q]
    return np.ascontiguousarray(
        xt2d.reshape(EC, 128, nchunks, 512).transpose(2, 1, 0, 3))


def _tile_w(wt_slice):
    # [E, 128] -> [128, EC, 128]
    return np.ascontiguousarray(
        wt_slice.reshape(EC, 128, 128).transpose(1, 0, 2))


def make_in_maps(x1, x2, Wq, bq, Wk, bk, Wv, bv, Wo, bo=None):
    x1 = np.asarray(x1, dtype=np.float32)
    x2 = np.asarray(x2, dtype=np.float32)
    x1t = _tile_x(np.ascontiguousarray(x1.reshape(Q_ROWS, E).T), QC)
    x2t = _tile_x(np.ascontiguousarray(x2.reshape(KV_ROWS, E).T),
                  KV_ROWS // 512)
    WqT = np.asarray(Wq, dtype=np.float32).T
    WkT = np.asarray(Wk, dtype=np.float32).T
    WvT = np.asarray(Wv, dtype=np.float32).T
    WoT = np.ascontiguousarray(np.asarray(Wo, dtype=np.float32).T)
    ident = np.eye(128, dtype=np.float32)
    ones = np.ones((128, 1), dtype=np.float32)
    in_maps = []
    for c in range(N_CORES):
        s = slice(128 * c, 128 * (c + 1))
        in_maps.append({
            "x1t": x1t, "x2t": x2t,
            "wqt": _tile_w(WqT[:, s]),
            "wkt": _tile_w(WkT[:, s]),
            "wvt": _tile_w(WvT[:, s]),
            "wot": np.ascontiguousarray(WoT[s, :]),
            "bq": np.ascontiguousarray(
                np.asarray(bq, np.float32)[s]).reshape(128, 1),
            "bk": np.ascontiguousarray(
                np.asarray(bk, np.float32)[s]).reshape(128, 1),
            "bv": np.ascontiguousarray(
                np.asarray(bv, np.float32)[s]).reshape(128, 1),
            "ident": ident, "ones": ones,
        })
    return in_maps


def kernel(x1, x2, Wq, bq, Wk, bk, Wv, bv, Wo, bo):
    nc = _get_nc()
    in_maps = make_in_maps(x1, x2, Wq, bq, Wk, bk, Wv, bv, Wo)
    res = run_bass_kernel_spmd(nc, in_maps, list(range(N_CORES)))
    ytf = res.results[0]["yt"].astype(np.float64)
    for c in range(1, N_CORES):
        ytf += res.results[c]["yt"]
    y = ytf.T.astype(np.float32) + np.asarray(bo, np.float32)[None, :]
    return y.reshape(B, SQ, E)
